# revision 17
# baseline (speedup 1.0000x reference)
"""Trainium2 Bass kernel for nn_Decoder (degenerate LSTM decoder).

Math (see reference):
  gates = x @ W_ih^T + (b_ih + b_hh)      [B, T, 4D], gate order i, f, g, o
  c = sigmoid(i) * tanh(g)                (f unused: c0 = 0)
  h = sigmoid(o) * tanh(c)                [B, T, D]
  out = softmax((h.reshape(B, T*D) @ W_out^T + b_out).reshape(B, 4, 10), axis=2)

Strategy: pure data parallel over 8 cores (batch 2048 -> 256/core).
Per core, batch stays on SBUF partitions everywhere:
  - host pre-transposes x to [d+1, btile, t, b] fp16 (ones row folds the bias
    into the K=91 contraction), so mm1 is lhsT=x_t^T [91,128], rhs=W [91,270]
    -> gates [128b, 270] in PSUM (col order i, o, g; g pre-scaled by 2).
  - ACT applies a single Sigmoid to all 270 cols straight from PSUM:
    si = sig(i), so = sig(o), sg2 = sig(2g) = (tanh(g)+1)/2.  That's the only
    per-element ACT work (3 evals); tanh(c) is replaced by the minimax cubic
    tanh(c) ~ c*(CA + CB*c^2) (max err 3.9e-3 on |c|<=0.97) computed on DVE:
      tgx = 2*sg2 - 1            (tensor_scalar 4x)   = tanh(g)
      c   = si * tgx             (tensor_tensor 2x)
      ss  = c * c                (tensor_tensor 2x)
      qq  = CB*ss + CA           (tensor_scalar 4x)
      rr  = qq * c               (tensor_tensor 2x)
      h   = rr * so              (tensor_tensor 2x)
  - gates PSUM is split 4 banks (btile0) + 3 banks (btile1) + 1 bank logits,
    so ACT consumes 4-slot / 3-slot groups while mm1 refills the other slots.
  - h [128, 21632] fp16 is transposed in 128-wide td-chunks by the DMA xbar;
    mm2 accumulates logitsT [40, 128] over 169 chunks in PSUM.
  - logitsT + b_out, PE-transpose to [128, 40], exp with per-group accum sums,
    reciprocal, scale -> softmax; store fp32.
"""

import numpy as np
from contextlib import ExitStack

import concourse.bass as bass
import concourse.bacc as bacc
import concourse.tile as tile
from concourse import mybir
from concourse.bass_utils import run_bass_kernel_spmd

F16 = mybir.dt.float16
F32 = mybir.dt.float32
AF = mybir.ActivationFunctionType
OP = mybir.AluOpType

B, T, D = 2048, 240, 90
NCLS, NGRP, GRP = 40, 4, 10
NCORES = 8
BC = B // NCORES            # 256 batch rows per core
NBT = 2                     # 128-row btiles per core
K1 = D + 1                  # 91 = d + ones row (bias)
NG = 3 * D                  # 270 gate cols: i(90) o(90) g(90)
TD = T * D                  # 21600
NCH = (TD + 127) // 128     # 169
TDP = NCH * 128             # 21632
TBLK = 12                   # t-block for DVE/elementwise staging
GS = (4, 3)                 # gates PSUM ring depth per btile (4 + 3 banks)
TGRP = 13                   # td-chunks per batched DMA transpose (13*13=169)

# tanh(c) ~ c*(CA + CB*c^2) minimax on |c| <= 0.97 (max err 3.9e-3)
CA = 0.978225
CB = -0.224270

_CACHE: dict = {}
DEBUG_DUMP = False


def _build_nc():
    nc = bacc.Bacc("TRN2", target_bir_lowering=False, debug=False)

    xt_d = nc.dram_tensor("xt", [K1, NBT, T, 128], F16, kind="ExternalInput")
    wmov_d = nc.dram_tensor("wmov", [K1, NG], F16, kind="ExternalInput")
    wout_d = nc.dram_tensor("woutT", [128, NCH, NCLS], F16, kind="ExternalInput")
    bout_d = nc.dram_tensor("bout", [NCLS, 128], F32, kind="ExternalInput")
    id40_d = nc.dram_tensor("id40", [NCLS, NCLS], F32, kind="ExternalInput")
    out_d = nc.dram_tensor("out", [BC, NCLS], F32, kind="ExternalOutput")
    if DEBUG_DUMP:
        hdbg_d = nc.dram_tensor("hdbg", [128, TDP], F16, kind="ExternalOutput")
        tgadbg_d = nc.dram_tensor("tgadbg", [128, TBLK, NG], F16,
                                  kind="ExternalOutput")

    with ExitStack() as ctx:
        tc = ctx.enter_context(tile.TileContext(nc))
        consts = ctx.enter_context(tc.tile_pool(name="consts", bufs=1))
        xt_pool = ctx.enter_context(tc.tile_pool(name="xt", bufs=6))
        tga_pool = ctx.enter_context(tc.tile_pool(name="tga", bufs=4))
        dve_pool = ctx.enter_context(tc.tile_pool(name="dve", bufs=2))
        h_pool = ctx.enter_context(tc.tile_pool(name="h", bufs=2))
        ht_pool = ctx.enter_context(tc.tile_pool(name="ht", bufs=5))
        fin_pool = ctx.enter_context(tc.tile_pool(name="fin", bufs=2))
        pg_pool = ctx.enter_context(tc.tile_pool(name="pg", bufs=2, space="PSUM"))
        pl_pool = ctx.enter_context(tc.tile_pool(name="pl", bufs=1, space="PSUM"))

        wmov = consts.tile([K1, NG], F16)
        nc.sync.dma_start(out=wmov[:], in_=wmov_d[:])
        bout = consts.tile([NCLS, 128], F32)
        nc.sync.dma_start(out=bout[:], in_=bout_d[:])
        id40 = consts.tile([NCLS, NCLS], F32)
        nc.sync.dma_start(out=id40[:], in_=id40_d[:])

        # Both btiles are interleaved in the t-loop: two independent
        # mm1 <-> sigmoid PSUM dependency chains keep PE and ACT saturated.
        # Both logitsT tiles share one PSUM bank.  bt0's first mm2 uses
        # start=True (clears has_written for the whole bank) and runs before
        # bt1's first mm2, which therefore uses start=False: with its
        # has_written bits cleared it overwrites, exactly like a start.
        logT = pl_pool.tile([NCLS, NBT, 128], F32, name="logT")
        logTs = [logT[:, bt, :] for bt in range(NBT)]
        hs, gates_bt = [], []
        for bt in range(NBT):
            h = h_pool.tile([128, TDP], F16, tag="h")
            # zero the tail so the padded td-chunk contributes nothing
            nc.vector.memset(h[:, TD:TDP], 0.0)
            hs.append(h)
            gates_bt.append(
                pg_pool.tile([128, GS[bt], 512], F32, tag=f"g{bt}",
                             name=f"gates{bt}", bufs=1)
            )

        mm2_pend = [[], []]
        grp = [0, 0]

        def transpose_group(bt):
            g = grp[bt]
            ht = ht_pool.tile([128, TGRP, 128], F16, tag="ht")
            c0 = g * TGRP
            nc.sync.dma_start(
                out=ht[:],
                in_=hs[bt][:, c0 * 128:(c0 + TGRP) * 128],
                transpose=True,
            )
            mm2_pend[bt].append((g, ht))
            grp[bt] = g + 1

        def mm2_flush(bt, keep):
            # emit mm2 matmuls lagging the transposes so they never
            # stall at the head of the PE queue
            while len(mm2_pend[bt]) > keep:
                g, ht = mm2_pend[bt].pop(0)
                for i in range(TGRP):
                    ck = g * TGRP + i
                    nc.tensor.matmul(
                        logTs[bt],
                        wout[:, ck, :],
                        ht[:, i, :],
                        start=(ck == 0 and bt == 0),
                        stop=(ck == NCH - 1),
                        skip_group_check=True,
                    )

        # xt loads go through the single software-DGE queue, which the h
        # transposes also use.  A transpose waiting on h blocks later queue
        # entries, so xt loads are issued PFD blocks ahead of the transposes
        # that could block them.
        PFD = 2
        NTB = T // TBLK

        def load_xt(tb):
            tiles = []
            for bt in range(NBT):
                xt = xt_pool.tile([K1, TBLK, 128], F16, tag="xt",
                                  name=f"xt{tb}_{bt}", bufs=2 * (PFD + 1))
                # hardware DGE (sync queue): a separate DMA path from the
                # software DGE that executes the h transposes, so neither
                # blocks the other.
                nc.sync.dma_start(
                    out=xt[:], in_=xt_d[:, bt, tb * TBLK:(tb + 1) * TBLK, :]
                )
                tiles.append(xt)
            return tiles

        xt_pre = {tb: load_xt(tb) for tb in range(PFD)}
        # wout (1.7 MB) is only needed by mm2; load it after the first xt
        # blocks so it doesn't delay the first mm1 matmuls.
        wout = consts.tile([128, NCH, NCLS], F16)
        nc.sync.dma_start(out=wout[:], in_=wout_d[:])
        for tb in range(NTB):
            if tb + PFD < NTB:
                xt_pre[tb + PFD] = load_xt(tb + PFD)
            xts = xt_pre.pop(tb)
            # tga = sigmoid of all gate cols: [sig(i), sig(o), sig(2g)]
            tgas = [
                tga_pool.tile([128, TBLK, NG], F16, tag=f"tga{bt}",
                              name=f"tga{bt}", bufs=2)
                for bt in range(NBT)
            ]
            for j in range(TBLK):
                t = tb * TBLK + j
                for bt in range(NBT):
                    s = t % GS[bt]
                    nc.tensor.matmul(
                        gates_bt[bt][:, s, 0:NG],
                        xts[bt][:, j, :],
                        wmov[:],
                        start=True,
                        stop=True,
                        skip_group_check=True,
                    )
                    if s == GS[bt] - 1:
                        nc.scalar.activation(
                            tgas[bt][:, j - s:j + 1, :],
                            gates_bt[bt][:, 0:GS[bt], 0:NG],
                            AF.Sigmoid,
                        )
            # DVE: h = sig(o) * c * (CA + CB*c^2),  c = sig(i)*tanh(g),
            # tanh(g) = 2*sig(2g) - 1.
            for bt in range(NBT):
                tga = tgas[bt]
                si = tga[:, :, 0:D]
                so = tga[:, :, D:2 * D]
                sg2 = tga[:, :, 2 * D:NG]
                tgx = dve_pool.tile([128, TBLK, D], F16, tag="tgx", bufs=2)
                cc = dve_pool.tile([128, TBLK, D], F16, tag="cc", bufs=2)
                ss = dve_pool.tile([128, TBLK, D], F16, tag="ss", bufs=2)
                qq = dve_pool.tile([128, TBLK, D], F16, tag="qq", bufs=2)
                rr = dve_pool.tile([128, TBLK, D], F16, tag="rr", bufs=2)
                nc.vector.tensor_scalar(
                    out=tgx[:], in0=sg2, scalar1=2.0, scalar2=-1.0,
                    op0=OP.mult, op1=OP.add,
                )
                nc.vector.tensor_mul(cc[:], si, tgx[:])
                nc.vector.tensor_mul(ss[:], cc[:], cc[:])
                nc.vector.tensor_scalar(
                    out=qq[:], in0=ss[:], scalar1=CB, scalar2=CA,
                    op0=OP.mult, op1=OP.add,
                )
                nc.vector.tensor_mul(rr[:], qq[:], cc[:])
                hv = hs[bt][:, tb * TBLK * D:(tb + 1) * TBLK * D].rearrange(
                    "p (t d) -> p t d", d=D
                )
                nc.vector.tensor_mul(hv, rr[:], so)
                # drain completed groups of TGRP td-chunks into mm2.
                # keep=2: mm2 matmuls lag their transposes by two groups so
                # they never reach the head of the FIFO PE queue before the
                # DMA xbar has produced their ht input (a waiting mm2 matmul
                # would block the mm1 matmuls that feed ACT).
                while (grp[bt] + 1) * TGRP * 128 <= (tb + 1) * TBLK * D:
                    transpose_group(bt)
                    mm2_flush(bt, keep=1)
            if DEBUG_DUMP and tb == 0:
                nc.sync.dma_start(out=tgadbg_d[:], in_=tgas[0][:])
        if DEBUG_DUMP:
            nc.sync.dma_start(out=hdbg_d[:], in_=hs[0][:])

        # tail: finish bt's mm2 chain then run its softmax while the other
        # btile's tail mm2 matmuls are still draining
        for bt in range(NBT):
            while grp[bt] < NCH // TGRP:  # tail group (includes the zero pad)
                transpose_group(bt)
            mm2_flush(bt, keep=0)
            lsb = fin_pool.tile([NCLS, 128], F32)
            nc.vector.tensor_add(lsb[:], logTs[bt], bout[:])
            smax = pg_pool.tile([128, NCLS], F32, tag="g0", bufs=1)
            nc.tensor.transpose(smax[:], lsb[:], id40[:])
            esb = fin_pool.tile([128, NCLS], F32)
            sums = fin_pool.tile([128, NGRP], F32)
            for g in range(NGRP):
                nc.scalar.activation(
                    esb[:, g * GRP:(g + 1) * GRP],
                    smax[:, g * GRP:(g + 1) * GRP],
                    AF.Exp,
                    accum_out=sums[:, g:g + 1],
                )
            rcp = fin_pool.tile([128, NGRP], F32)
            nc.vector.reciprocal(rcp[:], sums[:])
            ob = fin_pool.tile([128, NCLS], F32)
            for g in range(NGRP):
                nc.scalar.activation(
                    ob[:, g * GRP:(g + 1) * GRP],
                    esb[:, g * GRP:(g + 1) * GRP],
                    AF.Copy,
                    scale=rcp[:, g:g + 1],
                )
            nc.sync.dma_start(out=out_d[bt * 128:(bt + 1) * 128, :], in_=ob[:])

    nc.compile()
    return nc


def get_nc():
    if "nc" not in _CACHE:
        _CACHE["nc"] = _build_nc()
    return _CACHE["nc"]


def make_in_maps(x, W_ih, W_hh, b_ih, b_hh, W_out, b_out):
    f16 = mybir.dt.np(F16)
    bias = (np.asarray(b_ih, np.float32) + np.asarray(b_hh, np.float32))

    # moving operand [91, 270]: cols [i, o, g]; row 90 carries the bias.
    # g is pre-scaled by 2 so a single Sigmoid covers all gate columns:
    # tanh(g) = 2*sig(2g) - 1.
    wmov = np.zeros((K1, NG), np.float32)
    wmov[0:D, 0:D] = np.asarray(W_ih)[0:D].T                  # i
    wmov[0:D, D:2 * D] = np.asarray(W_ih)[3 * D:4 * D].T      # o
    wmov[0:D, 2 * D:NG] = 2.0 * np.asarray(W_ih)[2 * D:3 * D].T  # 2g
    wmov[D, 0:D] = bias[0:D]
    wmov[D, D:2 * D] = bias[3 * D:4 * D]
    wmov[D, 2 * D:NG] = 2.0 * bias[2 * D:3 * D]
    wmov = wmov.astype(f16)

    wt = np.zeros((TDP, NCLS), np.float32)
    wt[0:TD] = np.asarray(W_out, np.float32).T
    wout = np.ascontiguousarray(
        wt.reshape(NCH, 128, NCLS).transpose(1, 0, 2)
    ).astype(f16)

    boutr = np.ascontiguousarray(
        np.broadcast_to(np.asarray(b_out, np.float32)[:, None], (NCLS, 128))
    )
    id40 = np.eye(NCLS, dtype=np.float32)

    # x [2048, 240, 90] -> per core [91, btile, t, b] fp16 with ones row
    xs = np.asarray(x, np.float32).reshape(NCORES, NBT, 128, T, D)
    xt_all = np.empty((NCORES, K1, NBT, T, 128), f16)
    xt_all[:, 0:D] = xs.transpose(0, 4, 1, 3, 2).astype(f16)
    xt_all[:, D] = np.array(1.0, f16)

    return [
        {
            "xt": np.ascontiguousarray(xt_all[c]),
            "wmov": wmov,
            "woutT": wout,
            "bout": boutr,
            "id40": id40,
        }
        for c in range(NCORES)
    ]


def kernel(x, W_ih, W_hh, b_ih, b_hh, W_out, b_out, trace=False, **run_kwargs):
    nc = get_nc()
    in_maps = make_in_maps(x, W_ih, W_hh, b_ih, b_hh, W_out, b_out)
    res = run_bass_kernel_spmd(
        nc, in_maps, list(range(NCORES)), trace=trace, **run_kwargs
    )
    out = np.concatenate([res.results[c]["out"] for c in range(NCORES)], axis=0)
    out = out.reshape(B, NGRP, GRP).astype(np.float32)
    if trace:
        kernel.last_result = res
    return out
